# revision 1
# baseline (speedup 1.0000x reference)
"""TRN2 Bass kernel for nn_MultiHeadAttention_670014898403.

Takes the FULL unsharded inputs (as produced by setup_inputs()), shards
across 8 NeuronCores (2 batches x 4 head-groups of 4 heads / e-slice 256),
runs one SPMD Bass/Tile kernel per core, and gathers the full output.

Per-core device dataflow (all matmuls bf16 with fp32 PSUM accumulation):
  QT[e,t] = wqT.T @ xqT   (scale 1/sqrt(64) folded into wqT/bq on host)
  KT[e,t] = wkT.T @ xkT
  V[t,e]  = xvT.T @ wvT   (stored per t-block with a ones column per head)
  per head:  S^T = KT_h-block.T @ QT_h  (PSUM f32)
             P^T = exp(S^T)             (ScalarE, bf16, no max subtraction:
                                         |S| <= ~10 for this distribution)
             O^T[65, i] += V_aug.T @ P^T  (row 64 accumulates the softmax
                                           denominator via the ones column)
             A_h = O^T[0:64] * reciprocal(denom)
  outT[f,t] = woT.T @ A + bo   (bf16 partial, summed across cores on host)
"""
from contextlib import ExitStack

import numpy as np
import ml_dtypes

F32 = None
BF16 = None

T = 2048          # sequence length
D = 1024          # d_model
E = 256           # per-core projection width (4 heads x 64)
HPC = 4           # heads per core
DH = 64           # head dim
KB = D // 128     # contraction blocks for projections
TB = T // 128     # t-blocks / j-blocks
IC = 1024         # attention i-chunk
NIC = T // IC
B = 2
NCORES = 8
GPB = 4           # head-groups per batch
NP_BF16 = ml_dtypes.bfloat16

_CACHE = {}


def _dtypes():
    global F32, BF16
    from concourse import mybir
    F32 = mybir.dt.float32
    BF16 = mybir.dt.bfloat16


def _build_nc():
    import concourse.tile as tile
    from concourse import bacc
    _dtypes()
    nc = bacc.Bacc("TRN2", target_bir_lowering=False, debug=False,
                   enable_asserts=False, num_devices=NCORES)
    din = {}
    for name in ("xqT", "xkT", "xvT"):
        din[name] = nc.dram_tensor(name, [D, T], BF16, kind="ExternalInput").ap()
    for name in ("wqT", "wkT", "wvT"):
        din[name] = nc.dram_tensor(name, [D, E], BF16, kind="ExternalInput").ap()
    din["woT"] = nc.dram_tensor("woT", [E, D], BF16, kind="ExternalInput").ap()
    din["bq"] = nc.dram_tensor("bq", [E], F32, kind="ExternalInput").ap()
    din["bk"] = nc.dram_tensor("bk", [E], F32, kind="ExternalInput").ap()
    din["bv"] = nc.dram_tensor("bv", [E], F32, kind="ExternalInput").ap()
    din["bo"] = nc.dram_tensor("bo", [D], F32, kind="ExternalInput").ap()
    outT = nc.dram_tensor("outT", [D, T], BF16, kind="ExternalOutput").ap()
    with tile.TileContext(nc) as tc:
        _body(tc, nc, din, outT)
    nc.compile()
    return nc


def _body(tc, nc, din, outT):
    with ExitStack() as ctx:
        per = ctx.enter_context(tc.tile_pool(name="per", bufs=1))
        xq = per.tile([128, KB, T], BF16, tag="xq")
        xk = per.tile([128, KB, T], BF16, tag="xk")
        xv = per.tile([128, KB, T], BF16, tag="xv")
        wq = per.tile([128, KB, E], BF16, tag="wq")
        wk = per.tile([128, KB, E], BF16, tag="wk")
        wv = per.tile([128, KB, E], BF16, tag="wv")
        wo = per.tile([128, 2, D], BF16, tag="wo")
        bq = per.tile([128, 2], F32, tag="bq")
        bk = per.tile([128, 2], F32, tag="bk")
        bvb = per.tile([128, E], F32, tag="bvb")
        bo = per.tile([128, KB], F32, tag="bo")
        qt = per.tile([128, 2, T], BF16, tag="qt")
        kt = per.tile([128, 2, T], BF16, tag="kt")
        vv = per.tile([128, TB, HPC * (DH + 1)], BF16, tag="vv")
        aa = per.tile([128, 2, T], BF16, tag="aa")

        # biases first: they gate the PSUM->SBUF casts
        nc.sync.dma_start(bq[:], din["bq"].rearrange("(a p) -> p a", p=128))
        nc.sync.dma_start(bk[:], din["bk"].rearrange("(a p) -> p a", p=128))
        nc.sync.dma_start(bo[:], din["bo"].rearrange("(a p) -> p a", p=128))
        nc.sync.dma_start(bvb[:], din["bv"].partition_broadcast(128))
        for wname, wdst, xname, xdst in (("wkT", wk, "xkT", xk),
                                         ("wvT", wv, "xvT", xv),
                                         ("wqT", wq, "xqT", xq)):
            nc.sync.dma_start(wdst[:],
                              din[wname].rearrange("(kb p) t -> p kb t", p=128))
            src = din[xname].rearrange("(kb p) t -> p kb t", p=128)
            for kb in range(KB):
                nc.sync.dma_start(xdst[:, kb, :], src[:, kb, :])
        nc.sync.dma_start(wo[:], din["woT"].rearrange("(kb p) t -> p kb t", p=128))

        nc.vector.memset(vv[:], 1.0)  # ones columns for the denominator

        _proj(tc, nc, (xq, xk, xv), (wq, wk, wv), (bq, bk, bvb), qt, kt, vv)

        with tc.tile_pool(name="psS", bufs=2, space="PSUM") as psS, \
             tc.tile_pool(name="psO", bufs=4, space="PSUM") as psO, \
             tc.tile_pool(name="sbP", bufs=3) as sbP, \
             tc.tile_pool(name="sbN", bufs=4) as sbN:
            _attn(nc, psS, psO, sbP, sbN, qt, kt, vv, aa)

        with tc.tile_pool(name="psC", bufs=4, space="PSUM") as psC, \
             tc.tile_pool(name="sbO", bufs=4) as sbO:
            _oproj(nc, psC, sbO, wo, bo, aa, outT)


def _proj(tc, nc, xs, ws, bs, qt, kt, vv):
    xq, xk, xv = xs
    wq, wk, wv = ws
    bq, bk, bvb = bs
    # kb-OUTER with 8 live psum groups so PE tracks per-kb DMA arrivals.
    with tc.tile_pool(name="psK", bufs=8, space="PSUM") as psK:
        pss = [psK.tile([128, 512], F32, tag="pp", name=f"pp_k_{g}")
               for g in range(8)]
        for kb in range(KB):
            for et in range(2):
                for nch in range(4):
                    nc.tensor.matmul(
                        pss[et * 4 + nch][:],
                        wk[:, kb, et * 128:(et + 1) * 128],
                        xk[:, kb, nch * 512:(nch + 1) * 512],
                        start=(kb == 0), stop=(kb == KB - 1))
        for et in range(2):
            for nch in range(4):
                nc.vector.tensor_scalar_add(
                    kt[:, et, nch * 512:(nch + 1) * 512],
                    pss[et * 4 + nch][:], bk[:, et:et + 1])

        for wave in range(2):
            tbs = list(range(wave * 8, wave * 8 + 8))
            psv = [psK.tile([128, 512], F32, tag="pp", name=f"vp_{tb}")
                   for tb in tbs]
            for kb in range(KB):
                for i, tb in enumerate(tbs):
                    nc.tensor.matmul(
                        psv[i][:, 0:E],
                        xv[:, kb, tb * 128:(tb + 1) * 128],
                        wv[:, kb, :],
                        start=(kb == 0), stop=(kb == KB - 1))
            for i, tb in enumerate(tbs):
                for h in range(HPC):
                    nc.vector.tensor_add(
                        vv[:, tb, h * (DH + 1): h * (DH + 1) + DH],
                        psv[i][:, h * DH:(h + 1) * DH],
                        bvb[:, h * DH:(h + 1) * DH])

        psq = [psK.tile([128, 512], F32, tag="pp", name=f"pp_q_{g}")
               for g in range(8)]
        for kb in range(KB):
            for et in range(2):
                for nch in range(4):
                    nc.tensor.matmul(
                        psq[et * 4 + nch][:],
                        wq[:, kb, et * 128:(et + 1) * 128],
                        xq[:, kb, nch * 512:(nch + 1) * 512],
                        start=(kb == 0), stop=(kb == KB - 1))
        for et in range(2):
            for nch in range(4):
                nc.vector.tensor_scalar_add(
                    qt[:, et, nch * 512:(nch + 1) * 512],
                    psq[et * 4 + nch][:], bq[:, et:et + 1])


def _attn(nc, psS, psO, sbP, sbN, qt, kt, vv, aa):
    from concourse import mybir
    Exp = mybir.ActivationFunctionType.Exp
    for h in range(HPC):
        et, eo = h // 2, (h % 2) * 64
        for ic in range(NIC):
            oc = [psO.tile([DH + 1, 512], F32, tag="O", name=f"oc{h}_{ic}_{i}")
                  for i in range(IC // 512)]
            for jb in range(TB):
                st = psS.tile([128, IC], F32, tag="S")
                for nn in range(IC // 512):
                    nc.tensor.matmul(
                        st[:, nn * 512:(nn + 1) * 512],
                        kt[eo:eo + DH, et, jb * 128:(jb + 1) * 128],
                        qt[eo:eo + DH, et,
                           ic * IC + nn * 512: ic * IC + (nn + 1) * 512],
                        start=True, stop=True)
                pt = sbP.tile([128, IC], BF16, tag="P")
                nc.scalar.activation(pt[:], st[:], Exp)
                for nn in range(IC // 512):
                    nc.tensor.matmul(
                        oc[nn][:],
                        vv[:, jb, h * (DH + 1):(h + 1) * (DH + 1)],
                        pt[:, nn * 512:(nn + 1) * 512],
                        start=(jb == 0), stop=(jb == TB - 1))
            for nn in range(IC // 512):
                base = ic * IC + nn * 512
                ops = oc[nn]
                dcp = sbN.tile([1, 512], F32, tag="dcp")
                nc.vector.tensor_copy(dcp[:], ops[DH:DH + 1, :])
                rr = sbN.tile([1, 512], F32, tag="rr")
                nc.vector.reciprocal_approx_fast(rr[:], dcp[:])
                rb = sbN.tile([DH, 512], F32, tag="rb")
                nc.gpsimd.partition_broadcast(rb[:], rr[:])
                nc.vector.tensor_mul(aa[eo:eo + DH, et, base:base + 512],
                                     ops[0:DH, :], rb[:])


def _oproj(nc, psC, sbO, wo, bo, aa, outT):
    from concourse import mybir
    Ident = mybir.ActivationFunctionType.Identity
    for ft in range(KB):
        stg = sbO.tile([128, T], BF16, tag="stg")
        for nch in range(4):
            ps = psC.tile([128, 512], F32, tag="op")
            for kb in range(2):
                nc.tensor.matmul(
                    ps[:],
                    wo[:, kb, ft * 128:(ft + 1) * 128],
                    aa[:, kb, nch * 512:(nch + 1) * 512],
                    start=(kb == 0), stop=(kb == 1))
            dst = stg[:, nch * 512:(nch + 1) * 512]
            if nch % 2 == 0:
                nc.vector.tensor_scalar_add(dst, ps[:], bo[:, ft:ft + 1])
            else:
                nc.scalar.activation(dst, ps[:], Ident, bias=bo[:, ft:ft + 1])
        nc.sync.dma_start(
            outT.rearrange("(ft p) t -> p ft t", p=128)[:, ft, :], stg[:])


def _core_inputs(c, q, k, v, Wq, bq, Wk, bk, Wv, bv, Wo, bo):
    b, g = divmod(c, GPB)
    es = slice(g * E, g * E + E)
    return {
        "xqT": np.ascontiguousarray(q[b].T).astype(NP_BF16),
        "xkT": np.ascontiguousarray(k[b].T).astype(NP_BF16),
        "xvT": np.ascontiguousarray(v[b].T).astype(NP_BF16),
        "wqT": np.ascontiguousarray((Wq[es, :] / 8.0).T).astype(NP_BF16),
        "wkT": np.ascontiguousarray(Wk[es, :].T).astype(NP_BF16),
        "wvT": np.ascontiguousarray(Wv[es, :].T).astype(NP_BF16),
        "woT": np.ascontiguousarray(Wo[:, es].T).astype(NP_BF16),
        "bq": (bq[es] / 8.0).astype(np.float32),
        "bk": bk[es].astype(np.float32),
        "bv": bv[es].astype(np.float32),
        "bo": (bo if g == 0 else np.zeros_like(bo)).astype(np.float32),
    }


def kernel(q, k, v, Wq, bq, Wk, bk, Wv, bv, Wo, bo):
    from concourse.bass_utils import run_bass_kernel_spmd

    if "nc" not in _CACHE:
        _CACHE["nc"] = _build_nc()
    nc = _CACHE["nc"]

    args = dict(q=np.asarray(q, np.float32), k=np.asarray(k, np.float32),
                v=np.asarray(v, np.float32), Wq=np.asarray(Wq, np.float32),
                bq=np.asarray(bq, np.float32), Wk=np.asarray(Wk, np.float32),
                bk=np.asarray(bk, np.float32), Wv=np.asarray(Wv, np.float32),
                bv=np.asarray(bv, np.float32), Wo=np.asarray(Wo, np.float32),
                bo=np.asarray(bo, np.float32))
    in_maps = [_core_inputs(c, **args) for c in range(NCORES)]
    res = run_bass_kernel_spmd(nc, in_maps, core_ids=list(range(NCORES)))
    out = np.zeros((B, T, D), np.float32)
    for c, r in enumerate(res.results):
        out[c // GPB] += r["outT"].T.astype(np.float32)
    return out


# revision 2
# speedup vs baseline: 55.2869x; 55.2869x over previous
"""Bass/Tile kernel builder for sharded MultiHeadAttention on TRN2.

Sharding: 8 cores = 2 batches x 4 head-groups (4 heads each, e-slice of 256).
Each core gets host-transposed bf16 inputs and computes a partial output
outT [1024, 2048] (bf16, transposed); host sums the 4 head-group partials
per batch and transposes back.

Device dataflow (per core):
  QT[e,t] = wqT.T @ xqT   (scale 1/8 folded into wqT/bq on host)
  KT[e,t] = wkT.T @ xkT
  V[t,e]  = xvT.T @ wvT   (per t-block with a ones column per head)
  per head:  S^T[j,i] = KT_h.T-block @ QT_h   (K=64, PSUM f32)
             P^T = exp(S^T)                   (ACT, bf16, no max subtraction)
             O^T[65,i] += V_aug.T @ P^T       (row 64 = softmax denominator)
             A_h = O^T[0:64] * reciprocal(denom)
  outT[f,t] = woT.T @ A + bo

Scheduling structure: x tensors rotate through 2 SBUF slots; projections run
kb-outer so PE tracks DMA arrivals; order K-proj, Q-proj, then QK+exp for
head 0 chunk 0 (ScalarE starts ASAP), then V-proj (through the O-tag PSUM
slots), then the PV for the prefetched chunk and the rest of the attention.
"""
from contextlib import ExitStack

import concourse.bass as bass
import concourse.tile as tile
from concourse import bacc, mybir

F32 = mybir.dt.float32
BF16 = mybir.dt.bfloat16

T = 2048          # sequence length
D = 1024          # d_model
E = 256           # per-core projection width (4 heads x 64)
HPC = 4           # heads per core
DH = 64           # head dim
KB = D // 128     # contraction blocks for projections
TB = T // 128     # t-blocks / j-blocks
IC = 1024         # attention i-chunk
NIC = T // IC
PRE_BUFS = 16     # P-tile slots (run-ahead for the prefetched chunk)
ALL_PHASES = ("proj", "attn", "oproj")


def build_nc(phases=ALL_PHASES, reps=1):
    nc = bacc.Bacc("TRN2", target_bir_lowering=False, debug=False,
                   enable_asserts=False, num_devices=8)
    din = {}
    for name in ("xqT", "xkT", "xvT"):
        din[name] = nc.dram_tensor(name, [D, T], BF16, kind="ExternalInput").ap()
    for name in ("wqT", "wkT", "wvT"):
        din[name] = nc.dram_tensor(name, [D, E], BF16, kind="ExternalInput").ap()
    din["woT"] = nc.dram_tensor("woT", [E, D], BF16, kind="ExternalInput").ap()
    din["bq"] = nc.dram_tensor("bq", [E], F32, kind="ExternalInput").ap()
    din["bk"] = nc.dram_tensor("bk", [E], F32, kind="ExternalInput").ap()
    din["bv"] = nc.dram_tensor("bv", [E], F32, kind="ExternalInput").ap()
    din["bo"] = nc.dram_tensor("bo", [D], F32, kind="ExternalInput").ap()
    outT = nc.dram_tensor("outT", [D, T], BF16, kind="ExternalOutput").ap()

    with tile.TileContext(nc) as tc:
        for _ in range(reps):
            _body(tc, nc, din, outT, phases=phases)
    nc.compile()
    return nc


def _body(tc, nc, din, outT, dbg=None, phases=ALL_PHASES):
    Exp = mybir.ActivationFunctionType.Exp
    with ExitStack() as ctx:
        per = ctx.enter_context(tc.tile_pool(name="per", bufs=1))

        # ---- persistent SBUF tensors ----
        wq = per.tile([128, KB, E], BF16, tag="wq")
        wk = per.tile([128, KB, E], BF16, tag="wk")
        wv = per.tile([128, KB, E], BF16, tag="wv")
        wo = per.tile([128, 2, D], BF16, tag="wo")
        bq = per.tile([128, 2], F32, tag="bq")
        bk = per.tile([128, 2], F32, tag="bk")
        bvb = per.tile([128, E], F32, tag="bvb")
        bo = per.tile([128, KB], F32, tag="bo")
        qt = per.tile([128, 2, T], BF16, tag="qt")    # [e within tile, et, t]
        kt = per.tile([128, 2, T], BF16, tag="kt")
        vv = per.tile([128, TB, HPC * (DH + 1)], BF16, tag="vv")
        aa = per.tile([128, 2, T], BF16, tag="aa")    # normalized attn out A^T

        if not phases or ("proj" not in phases and "attn" not in phases):
            # null body for overhead calibration
            with tc.tile_pool(name="sbO", bufs=1) as sbO:
                nc.sync.dma_start(bo[:], din["bo"].rearrange("(a p) -> p a", p=128))
                stg = sbO.tile([128, 8], BF16, tag="stgnull")
                nc.vector.tensor_copy(stg[:], bo[:])
                nc.sync.dma_start(
                    outT.rearrange("(ft p) t -> p ft t", p=128)[:, 0, 0:8], stg[:])
            return

        sbX = ctx.enter_context(tc.tile_pool(name="sbX", bufs=2))
        xk = sbX.tile([128, KB, T], BF16, tag="x", name="xk")
        xq = sbX.tile([128, KB, T], BF16, tag="x", name="xq")
        xv = sbX.tile([128, KB, T], BF16, tag="x", name="xv")

        # ---- input DMAs: tiny biases FIRST (they gate PSUM->SBUF casts),
        # then K, Q, V; xv reuses xk's slot after K-proj drains it ----
        nc.sync.dma_start(bq[:], din["bq"].rearrange("(a p) -> p a", p=128))
        nc.sync.dma_start(bk[:], din["bk"].rearrange("(a p) -> p a", p=128))
        nc.sync.dma_start(bo[:], din["bo"].rearrange("(a p) -> p a", p=128))
        nc.sync.dma_start(bvb[:], din["bv"].partition_broadcast(128))
        for wname, wdst, xname, xdst in (("wkT", wk, "xkT", xk),
                                         ("wqT", wq, "xqT", xq),
                                         ("wvT", wv, "xvT", xv)):
            nc.sync.dma_start(wdst[:],
                              din[wname].rearrange("(kb p) t -> p kb t", p=128))
            src = din[xname].rearrange("(kb p) t -> p kb t", p=128)
            for kb in range(KB):
                nc.sync.dma_start(xdst[:, kb, :], src[:, kb, :])
        nc.sync.dma_start(wo[:], din["woT"].rearrange("(kb p) t -> p kb t", p=128))

        # ones columns for the denominator trick
        nc.vector.memset(vv[:], 1.0)

        # ---- K and Q projections (kb-outer, 8 live psum groups) ----
        with tc.tile_pool(name="psK", bufs=8, space="PSUM") as psK:
            for pname, w_t, b_t, src, dst in (("k", wk, bk, xk, kt),
                                              ("q", wq, bq, xq, qt)):
                pss = [psK.tile([128, 512], F32, tag="pp", name=f"pp_{pname}{g}")
                       for g in range(8)]
                for kb in range(KB):
                    for et in range(2):
                        for nch in range(4):
                            nc.tensor.matmul(
                                pss[et * 4 + nch][:],
                                w_t[:, kb, et * 128:(et + 1) * 128],
                                src[:, kb, nch * 512:(nch + 1) * 512],
                                start=(kb == 0), stop=(kb == KB - 1))
                for et in range(2):
                    for nch in range(4):
                        nc.vector.tensor_scalar_add(
                            dst[:, et, nch * 512:(nch + 1) * 512],
                            pss[et * 4 + nch][:], b_t[:, et:et + 1])

        # ---- attention (+ V-proj interleaved through the O-tag slots) ----
        with tc.tile_pool(name="psS", bufs=2, space="PSUM") as psS, \
             tc.tile_pool(name="psO", bufs=4, space="PSUM") as psO, \
             tc.tile_pool(name="sbP", bufs=PRE_BUFS) as sbP, \
             tc.tile_pool(name="sbN", bufs=3) as sbN:

            def qk_exp(h, ic, jb):
                et, eo = h // 2, (h % 2) * 64
                st = psS.tile([128, IC], F32, tag="S", name=f"st{h}_{ic}_{jb}")
                for nn in range(IC // 512):
                    nc.tensor.matmul(
                        st[:, nn * 512:(nn + 1) * 512],
                        kt[eo:eo + DH, et, jb * 128:(jb + 1) * 128],
                        qt[eo:eo + DH, et,
                           ic * IC + nn * 512: ic * IC + (nn + 1) * 512],
                        start=True, stop=True)
                pt = sbP.tile([128, IC], BF16, tag="P", name=f"pt{h}_{ic}_{jb}")
                nc.scalar.activation(pt[:], st[:], Exp)
                if dbg is not None and h == 0 and ic == 0 and jb == 0:
                    nc.sync.dma_start(dbg["p0"], pt[:])
                return pt

            def pv(h, oc, jb, pt):
                for nn in range(IC // 512):
                    nc.tensor.matmul(
                        oc[nn][0:DH + 1, :],
                        vv[:, jb, h * (DH + 1):(h + 1) * (DH + 1)],
                        pt[:, nn * 512:(nn + 1) * 512],
                        start=(jb == 0), stop=(jb == TB - 1))

            def alloc_oc(h, ic):
                return [psO.tile([128, 512], F32, tag="O", name=f"oc{h}_{ic}_{i}")
                        for i in range(IC // 512)]

            def normalize(h, ic, oc):
                et, eo = h // 2, (h % 2) * 64
                for nn in range(IC // 512):
                    base = ic * IC + nn * 512
                    ops = oc[nn]
                    dcp = sbN.tile([1, 512], F32, tag="dcp")
                    nc.vector.tensor_copy(dcp[:], ops[DH:DH + 1, :])
                    rr = sbN.tile([1, 512], F32, tag="rr")
                    nc.vector.reciprocal_approx_fast(rr[:], dcp[:])
                    rb = sbN.tile([DH, 512], F32, tag="rb")
                    nc.gpsimd.partition_broadcast(rb[:], rr[:])
                    nc.vector.tensor_mul(aa[eo:eo + DH, et, base:base + 512],
                                         ops[0:DH, :], rb[:])
                    if dbg is not None:
                        nc.sync.dma_start(
                            dbg["dn"][h:h + 1, base:base + 512], dcp[:])
                        nc.sync.dma_start(
                            dbg["rr"][h:h + 1, base:base + 512], rr[:])

            # prefetched chunk: QK+exp for (h0, ic0) BEFORE V-proj so the
            # ScalarE pipeline starts as soon as Q-proj lands
            pts0 = [qk_exp(0, 0, jb) for jb in range(TB)]

            # V projection through the O-tag slots: 4 waves of 4 t-blocks
            for wave in range(4):
                tbs = list(range(wave * 4, wave * 4 + 4))
                psv = [psO.tile([128, 512], F32, tag="O", name=f"vp_{tb}")
                       for tb in tbs]
                for kb in range(KB):
                    for i, tb in enumerate(tbs):
                        nc.tensor.matmul(
                            psv[i][:, 0:E],
                            xv[:, kb, tb * 128:(tb + 1) * 128],
                            wv[:, kb, :],
                            start=(kb == 0), stop=(kb == KB - 1))
                for i, tb in enumerate(tbs):
                    for h in range(HPC):
                        nc.vector.tensor_add(
                            vv[:, tb, h * (DH + 1): h * (DH + 1) + DH],
                            psv[i][:, h * DH:(h + 1) * DH],
                            bvb[:, h * DH:(h + 1) * DH])

            # PV for the prefetched chunk
            oc = alloc_oc(0, 0)
            for jb in range(TB):
                pv(0, oc, jb, pts0[jb])
            normalize(0, 0, oc)

            # rest of the attention, standard interleave
            for h in range(HPC):
                for ic in range(NIC):
                    if h == 0 and ic == 0:
                        continue
                    oc = alloc_oc(h, ic)
                    for jb in range(TB):
                        pt = qk_exp(h, ic, jb)
                        pv(h, oc, jb, pt)
                    normalize(h, ic, oc)

        if dbg is not None:
            nc.sync.dma_start(dbg["qt"], qt[:].rearrange("p a t -> p (a t)"))
            nc.sync.dma_start(dbg["kt"], kt[:].rearrange("p a t -> p (a t)"))
            nc.sync.dma_start(dbg["vv"], vv[:].rearrange("p a t -> p (a t)"))
            nc.sync.dma_start(dbg["aa"], aa[:].rearrange("p a t -> p (a t)"))

        # ---- output projection ----
        if "oproj" in phases:
            with tc.tile_pool(name="psC", bufs=4, space="PSUM") as psC, \
                 tc.tile_pool(name="sbO", bufs=4) as sbO:
                _oproj(nc, psC, sbO, wo, bo, aa, outT)
        else:
            with tc.tile_pool(name="sbO", bufs=1) as sbO:
                stg = sbO.tile([128, 8], BF16, tag="stgnull")
                nc.vector.tensor_copy(stg[:], bo[:])
                nc.sync.dma_start(
                    outT.rearrange("(ft p) t -> p ft t", p=128)[:, 0, 0:8], stg[:])


def _oproj(nc, psC, sbO, wo, bo, aa, outT):
    Ident = mybir.ActivationFunctionType.Identity
    for ft in range(KB):  # 8 f-blocks of 128
        stg = sbO.tile([128, T], BF16, tag="stg")
        for nch in range(4):  # t chunks of 512
            ps = psC.tile([128, 512], F32, tag="op")
            for kb in range(2):
                nc.tensor.matmul(
                    ps[:],
                    wo[:, kb, ft * 128:(ft + 1) * 128],
                    aa[:, kb, nch * 512:(nch + 1) * 512],
                    start=(kb == 0), stop=(kb == 1))
            # alternate the PSUM->SBUF bias-add between DVE and ACT (ACT is
            # idle once the last exp has drained)
            dst = stg[:, nch * 512:(nch + 1) * 512]
            if nch % 2 == 0:
                nc.vector.tensor_scalar_add(dst, ps[:], bo[:, ft:ft + 1])
            else:
                nc.scalar.activation(dst, ps[:], Ident, bias=bo[:, ft:ft + 1])
        nc.sync.dma_start(
            outT.rearrange("(ft p) t -> p ft t", p=128)[:, ft, :], stg[:])


# ======================== host-side wrapper ========================
import numpy as np
import ml_dtypes

NP_BF16 = ml_dtypes.bfloat16
B = 2
NCORES = 8
GPB = 4
_CACHE = {}


def _core_inputs(c, q, k, v, Wq, bq, Wk, bk, Wv, bv, Wo, bo):
    b, g = divmod(c, GPB)
    es = slice(g * E, g * E + E)
    return {
        "xqT": np.ascontiguousarray(q[b].T).astype(NP_BF16),
        "xkT": np.ascontiguousarray(k[b].T).astype(NP_BF16),
        "xvT": np.ascontiguousarray(v[b].T).astype(NP_BF16),
        "wqT": np.ascontiguousarray((Wq[es, :] / 8.0).T).astype(NP_BF16),
        "wkT": np.ascontiguousarray(Wk[es, :].T).astype(NP_BF16),
        "wvT": np.ascontiguousarray(Wv[es, :].T).astype(NP_BF16),
        "woT": np.ascontiguousarray(Wo[:, es].T).astype(NP_BF16),
        "bq": (np.asarray(bq)[es] / 8.0).astype(np.float32),
        "bk": np.asarray(bk)[es].astype(np.float32),
        "bv": np.asarray(bv)[es].astype(np.float32),
        "bo": (np.asarray(bo) if g == 0 else np.zeros_like(bo)).astype(np.float32),
    }


def kernel(q, k, v, Wq, bq, Wk, bk, Wv, bv, Wo, bo):
    """Full-input MultiHeadAttention on 8 NeuronCores; returns [2,2048,1024] f32."""
    from concourse.bass_utils import run_bass_kernel_spmd

    if "nc" not in _CACHE:
        _CACHE["nc"] = build_nc()
    nc = _CACHE["nc"]

    args = dict(q=np.asarray(q, np.float32), k=np.asarray(k, np.float32),
                v=np.asarray(v, np.float32), Wq=np.asarray(Wq, np.float32),
                bq=np.asarray(bq, np.float32), Wk=np.asarray(Wk, np.float32),
                bk=np.asarray(bk, np.float32), Wv=np.asarray(Wv, np.float32),
                bv=np.asarray(bv, np.float32), Wo=np.asarray(Wo, np.float32),
                bo=np.asarray(bo, np.float32))
    in_maps = [_core_inputs(c, **args) for c in range(NCORES)]
    res = run_bass_kernel_spmd(nc, in_maps, core_ids=list(range(NCORES)))
    out = np.zeros((B, T, D), np.float32)
    for c, r in enumerate(res.results):
        out[c // GPB] += r["outT"].T.astype(np.float32)
    return out
